# revision 20
# baseline (speedup 1.0000x reference)
"""LoRA multi-head attention on 8 Trainium2 NeuronCores.

Sharding: data-parallel over batch (B=2) x tensor-parallel over heads
(16 heads -> 4 per core).  Core c handles batch b=c//4 and head group
g=c%4 (columns C=[256*g, 256*g+256) of the projection output).

Host prep (per weight): W_eff = W + 2.0 * B @ A  (exact LoRA fold),
and transposed activations x.T so the contraction dim lands on SBUF
partitions.  x and the QKV weights ship as bf16 (halves the DMA
prefix); attention tensors stay float32r (fp32 storage, full-rate PE).

Device, per core:
  K^T [256,2048], Q^T, V [2048,256] (per-head 65-wide, ones column
  folded in for softmax row sums), then per 512-wide q-tile x head
  pair: scoresT [t,q] (row-packed pair, one PSUM tile) -> exp (ACT,
  scale=1/8) -> PV -> pair-fused normalization -> ctx^T -> partial
  output projection (interleaved into the NEXT q-tile's PE stream;
  summed on host across the batch group).

Engine routing: input DMAs on Sync (never block - pure prefetch),
ctx/out DMAs on GpSimd (SWDGE), evictions + normalization on DVE,
exp on ACT.  PSUM (8 banks): "pj" 2, "sc" 2x[128,1024]=4, "ctx" 2.
"""

import sys

sys.path.insert(0, "/opt/trn_rl_repo")

from contextlib import ExitStack

import ml_dtypes
import numpy as np

import concourse.bass as bass
import concourse.tile as tile
from concourse import bacc, mybir
from concourse.bass_utils import run_bass_kernel_spmd

F32 = mybir.dt.float32
F32R = mybir.dt.float32r
BF16 = mybir.dt.bfloat16

B = 2
S = 2048
D = 1024
H = 16
DK = 64
SCALING = 2.0
N_CORES = 8
CPG = 4
CSLICE = D // CPG
Exp = mybir.ActivationFunctionType.Exp
MULT = mybir.AluOpType.mult

_CACHE = {}


def _build():
    nc = bacc.Bacc("TRN2", target_bir_lowering=False, debug=False)

    xqT = nc.declare_dram_parameter("xqT", [D, S], BF16, isOutput=False)
    xkT = nc.declare_dram_parameter("xkT", [D, S], BF16, isOutput=False)
    xvT = nc.declare_dram_parameter("xvT", [D, S], BF16, isOutput=False)
    wq = nc.declare_dram_parameter("wq", [D, CSLICE], BF16, isOutput=False)
    wk = nc.declare_dram_parameter("wk", [D, CSLICE], BF16, isOutput=False)
    wv = nc.declare_dram_parameter("wv", [D, CSLICE], BF16, isOutput=False)
    wo = nc.declare_dram_parameter("wo", [CSLICE, D], BF16, isOutput=False)
    outT = nc.declare_dram_parameter("outT", [D, S], F32, isOutput=True)

    with tile.TileContext(nc) as tc, ExitStack() as ctx:
        const = ctx.enter_context(tc.tile_pool(name="const", bufs=1))
        xp = ctx.enter_context(tc.tile_pool(name="xp", bufs=16))
        expp = ctx.enter_context(tc.tile_pool(name="expp", bufs=6))
        smallp = ctx.enter_context(tc.tile_pool(name="smallp", bufs=2))
        psum = ctx.enter_context(tc.tile_pool(name="psum", bufs=2, space="PSUM"))

        wq_sb = const.tile([128, 8, CSLICE], BF16)
        wk_sb = const.tile([128, 8, CSLICE], BF16)
        wv_sb = const.tile([128, 8, CSLICE], BF16)
        wo_sb = const.tile([128, 2, D], BF16)
        nc.sync.dma_start(wk_sb[:], wk.rearrange("(i p) c -> p i c", p=128))
        nc.sync.dma_start(wq_sb[:], wq.rearrange("(i p) c -> p i c", p=128))
        nc.sync.dma_start(wv_sb[:], wv.rearrange("(i p) c -> p i c", p=128))
        nc.sync.dma_start(wo_sb[:], wo.rearrange("(c p) o -> p c o", p=128))

        kT_s = [const.tile([128, 2, 512], BF16, name=f"kT{i}") for i in range(4)]
        qT_s = [const.tile([128, 2, 512], BF16, name=f"qT{i}") for i in range(4)]
        v_s = [const.tile([128, 4, 4, DK + 1], BF16, name=f"v{i}") for i in range(4)]
        ctxT_s = [const.tile([128, 2, 512], BF16, name=f"cx{i}") for i in range(4)]

        ones_f = const.tile([128, 16], F32)
        nc.vector.memset(ones_f[:], 1.0)
        for tt in range(4):
            nc.vector.tensor_copy(
                v_s[tt][:, :, :, DK : DK + 1],
                ones_f[:].rearrange("p (a b c) -> p a b c", a=4, b=4, c=1),
            )

        def proj_load(xsrc, st):
            xts = []
            for i in range(8):
                xt = xp.tile([128, 512], BF16, tag="xt")
                nc.sync.dma_start(
                    xt[:], xsrc[128 * i : 128 * (i + 1), 512 * st : 512 * (st + 1)]
                )
                xts.append(xt)
            return xts

        def proj_round(wsb, dst_tile, xts, cc):
            ps = psum.tile([128, 512], F32, tag="pj", bufs=2)
            for i in range(8):
                nc.tensor.matmul(
                    ps[:],
                    wsb[:, i, 128 * cc : 128 * (cc + 1)],
                    xts[i][:],
                    start=(i == 0),
                    stop=(i == 7),
                )
            nc.vector.tensor_copy(dst_tile[:, cc, :], ps[:])

        def proj_qk(xsrc, wsb, dst_tile, st):
            xts = proj_load(xsrc, st)
            for cc in range(2):
                proj_round(wsb, dst_tile, xts, cc)

        # ---- projections: K, Q(first tile), V; Q(rest) interleaves into
        # ---- the attention stream as PE filler ---------------------------
        for st in range(4):
            proj_qk(xkT, wk_sb, kT_s[st], st)
        proj_qk(xqT, wq_sb, qT_s[0], 0)
        for tt in range(4):
            xts = []
            for i in range(8):
                xt = xp.tile([128, 512], BF16, tag="xt")
                nc.sync.dma_start(
                    xt[:], xvT[128 * i : 128 * (i + 1), 512 * tt : 512 * (tt + 1)]
                )
                xts.append(xt)
            for t4 in range(4):
                ps = psum.tile([128, 256], F32, tag="pj", bufs=2)
                for i in range(8):
                    nc.tensor.matmul(
                        ps[:],
                        xts[i][:, 128 * t4 : 128 * (t4 + 1)],
                        wv_sb[:, i, :],
                        start=(i == 0),
                        stop=(i == 7),
                    )
                nc.vector.tensor_copy(
                    v_s[tt][:, t4, :, 0:DK],
                    ps[:].rearrange("p (h d) -> p h d", h=4),
                )

        # ---- attention; out-proj of qt-1 rides inside qt's p==1 loop ----
        ops_live = {}

        def outproj_emit(oqt, o, phase):
            """phase 0: start matmul; 1: stop matmul; 2: evict + store
            (emitted 2 t-steps after the stop so the DVE never stalls)."""
            if phase == 0:
                ops = psum.tile([128, 512], F32, tag="pj", bufs=2, name=f"op{oqt}_{o}")
                ops_live[(oqt, o)] = ops
                nc.tensor.matmul(
                    ops[:],
                    wo_sb[:, 0, 128 * o : 128 * (o + 1)],
                    ctxT_s[oqt][:, 0, :],
                    start=True,
                    stop=False,
                )
            elif phase == 1:
                ops = ops_live[(oqt, o)]
                nc.tensor.matmul(
                    ops[:],
                    wo_sb[:, 1, 128 * o : 128 * (o + 1)],
                    ctxT_s[oqt][:, 1, :],
                    start=False,
                    stop=True,
                )
            else:
                ops = ops_live.pop((oqt, o))
                ob = smallp.tile([128, 512], F32, tag="ob", bufs=3)
                nc.vector.tensor_copy(ob[:], ops[:])
                nc.sync.dma_start(
                    outT[128 * o : 128 * (o + 1), 512 * oqt : 512 * (oqt + 1)], ob[:]
                )

        qnext = [None]

        def attention(qt):
            qtile = qT_s[qt]
            for p in range(2):
                ctx0 = psum.tile([DK + 1, 512], F32, tag="ctx", bufs=2)
                ctx1 = psum.tile([DK + 1, 512], F32, tag="ctx", bufs=2)
                for t in range(16):
                    sc = psum.tile([128, 1024], F32, tag="sc", bufs=2)
                    kt = kT_s[t // 4]
                    ts_ = slice(128 * (t % 4), 128 * (t % 4 + 1))
                    nc.tensor.matmul(
                        sc[:, 0:512],
                        kt[0:64, p, ts_],
                        qtile[0:64, p, :],
                        start=True,
                        stop=True,
                        tile_position=(0, 0),
                    )
                    nc.tensor.matmul(
                        sc[:, 512:1024],
                        kt[64:128, p, ts_],
                        qtile[64:128, p, :],
                        start=True,
                        stop=True,
                        tile_position=(64, 0),
                    )
                    et = expp.tile([128, 1024], BF16)
                    nc.scalar.activation(et[:], sc[:], Exp, scale=1.0 / 8.0)
                    nc.tensor.matmul(
                        ctx0[:],
                        v_s[t // 4][:, t % 4, 2 * p, :],
                        et[:, 0:512],
                        start=(t == 0),
                        stop=(t == 15),
                    )
                    nc.tensor.matmul(
                        ctx1[:],
                        v_s[t // 4][:, t % 4, 2 * p + 1, :],
                        et[:, 512:1024],
                        start=(t == 0),
                        stop=(t == 15),
                    )
                    if qt > 0:
                        ob_base = 4 * p
                        if t in (4, 6, 8, 10):
                            outproj_emit(qt - 1, ob_base + (t - 4) // 2, 0)
                        if t in (5, 7, 9, 11):
                            outproj_emit(qt - 1, ob_base + (t - 5) // 2, 1)
                        if t in (7, 9, 11, 13):
                            outproj_emit(qt - 1, ob_base + (t - 7) // 2, 2)
                    if p == 0 and qt < 3:
                        if t == 3:
                            qnext[0] = proj_load(xqT, qt + 1)
                        elif t == 7:
                            proj_round(wq_sb, qT_s[qt + 1], qnext[0], 0)
                        elif t == 11:
                            proj_round(wq_sb, qT_s[qt + 1], qnext[0], 1)
                # evict both ctx psums FIRST (fast PSUM release keeps the
                # PE fed; a >3.4us PE idle re-throttles the HAM clock gate),
                # then run recip/bcast/mult off the critical path.
                css = []
                for cx in (ctx0, ctx1):
                    cs = smallp.tile([DK + 1, 512], F32, tag="cs", bufs=4)
                    nc.vector.tensor_copy(cs[:], cx[:])
                    css.append(cs)
                for h01, cs in enumerate(css):
                    rc = smallp.tile([1, 512], F32, tag="rc")
                    nc.vector.reciprocal(rc[:], cs[DK : DK + 1, :])
                    bc = smallp.tile([64, 512], F32, tag="bc")
                    nc.gpsimd.partition_broadcast(bc[:], rc[:])
                    ct = smallp.tile([64, 512], BF16, tag="ct")
                    nc.vector.tensor_tensor(ct[:], cs[0:DK, :], bc[:], MULT)
                    nc.sync.dma_start(ctxT_s[qt][64 * h01 : 64 * h01 + 64, p, :], ct[:])

        for qt in range(4):
            attention(qt)
        # tail: out-proj of the last q-tile
        for o in range(8):
            outproj_emit(3, o, 0)
            outproj_emit(3, o, 1)
        for o in range(8):
            outproj_emit(3, o, 2)

    nc.finalize()
    return nc


def _get_nc():
    if "nc" not in _CACHE:
        _CACHE["nc"] = _build()
    return _CACHE["nc"]


def _numpy_reference(query, key, value, mask, Wq, Aq, Bq, Wk, Ak, Bk, Wv, Av, Bv, Wo, Ao, Bo):
    """Exact fallback for a non-all-ones mask (never hit for the spec'd inputs)."""

    def lora(x, W, A, Bm):
        return x @ W.T + ((x @ A.T) @ Bm.T) * SCALING

    q = lora(query, Wq, Aq, Bq).reshape(B, S, H, DK).transpose(0, 2, 1, 3)
    k = lora(key, Wk, Ak, Bk).reshape(B, S, H, DK).transpose(0, 2, 1, 3)
    v = lora(value, Wv, Av, Bv).reshape(B, S, H, DK).transpose(0, 2, 1, 3)
    sc = np.einsum("bhqd,bhkd->bhqk", q, k) / np.sqrt(np.float32(DK))
    sc = np.where(mask == 0, np.float32(-1e9), sc)
    sc = sc - sc.max(axis=-1, keepdims=True)
    e = np.exp(sc)
    attn = e / e.sum(axis=-1, keepdims=True)
    cx = np.einsum("bhqk,bhkd->bhqd", attn, v)
    cx = cx.transpose(0, 2, 1, 3).reshape(B, S, D)
    return lora(cx, Wo, Ao, Bo).astype(np.float32)


def _prepare_in_maps(query, key, value, Wq, Aq, Bq, Wk, Ak, Bk, Wv, Av, Bv, Wo, Ao, Bo):
    f32 = np.float32
    bf16 = ml_dtypes.bfloat16
    weff = {}
    for n, (W, A, Bm) in {
        "q": (Wq, Aq, Bq),
        "k": (Wk, Ak, Bk),
        "v": (Wv, Av, Bv),
        "o": (Wo, Ao, Bo),
    }.items():
        weff[n] = (
            np.asarray(W, f32) + SCALING * np.asarray(Bm, f32) @ np.asarray(A, f32)
        ).astype(f32)

    xT = {
        "q": [np.ascontiguousarray(np.asarray(query[b], f32).T).astype(bf16) for b in range(B)],
        "k": [np.ascontiguousarray(np.asarray(key[b], f32).T).astype(bf16) for b in range(B)],
        "v": [np.ascontiguousarray(np.asarray(value[b], f32).T).astype(bf16) for b in range(B)],
    }
    in_maps = []
    for c in range(N_CORES):
        b, g = divmod(c, CPG)
        cs = slice(CSLICE * g, CSLICE * (g + 1))
        in_maps.append(
            {
                "xqT": xT["q"][b],
                "xkT": xT["k"][b],
                "xvT": xT["v"][b],
                "wq": np.ascontiguousarray(weff["q"][cs, :].T).astype(bf16),
                "wk": np.ascontiguousarray(weff["k"][cs, :].T).astype(bf16),
                "wv": np.ascontiguousarray(weff["v"][cs, :].T).astype(bf16),
                "wo": np.ascontiguousarray(weff["o"][:, cs].T).astype(bf16),
            }
        )
    return in_maps


def run(inputs, trace=False, **spmd_kwargs):
    """Shard, run on 8 cores, gather.  Returns (output, BassKernelResults)."""
    mask = np.asarray(inputs["mask"])
    if not np.all(mask != 0):
        out = _numpy_reference(
            np.asarray(inputs["query"], np.float32),
            np.asarray(inputs["key"], np.float32),
            np.asarray(inputs["value"], np.float32),
            mask,
            *[
                np.asarray(inputs[k], np.float32)
                for k in ("Wq", "Aq", "Bq", "Wk", "Ak", "Bk", "Wv", "Av", "Bv", "Wo", "Ao", "Bo")
            ],
        )
        return out, None

    in_maps = _prepare_in_maps(
        inputs["query"], inputs["key"], inputs["value"],
        inputs["Wq"], inputs["Aq"], inputs["Bq"],
        inputs["Wk"], inputs["Ak"], inputs["Bk"],
        inputs["Wv"], inputs["Av"], inputs["Bv"],
        inputs["Wo"], inputs["Ao"], inputs["Bo"],
    )
    nc = _get_nc()
    res = run_bass_kernel_spmd(
        nc, in_maps, core_ids=list(range(N_CORES)), trace=trace, **spmd_kwargs
    )
    out = np.empty((B, S, D), np.float32)
    for b in range(B):
        acc = res.results[CPG * b]["outT"].astype(np.float32)
        for g in range(1, CPG):
            acc = acc + res.results[CPG * b + g]["outT"]
        out[b] = acc.T
    return out, res


def kernel(**inputs):
    out, _ = run(inputs, trace=False)
    return out


# revision 22
# speedup vs baseline: 1.2030x; 1.2030x over previous
"""LoRA multi-head attention on 8 Trainium2 NeuronCores.

Sharding: data-parallel over batch (B=2) x tensor-parallel over heads
(16 heads -> 4 per core).  Core c handles batch b=c//4 and head group
g=c%4 (columns C=[256*g, 256*g+256) of the projection output).

Host prep (per weight): W_eff = W + 2.0 * B @ A  (exact LoRA fold),
and transposed activations x.T so the contraction dim lands on SBUF
partitions.  x and the QKV weights ship as bf16 (halves the DMA
prefix); attention tensors stay float32r (fp32 storage, full-rate PE).

Device, per core:
  K^T [256,2048], Q^T, V [2048,256] (per-head 65-wide, ones column
  folded in for softmax row sums), then per 512-wide q-tile x head
  pair: scoresT [t,q] (row-packed pair, one PSUM tile) -> exp (ACT,
  scale=1/8) -> PV -> pair-fused normalization -> ctx^T -> partial
  output projection (interleaved into the NEXT q-tile's PE stream;
  summed on host across the batch group).

Engine routing: input DMAs on Sync (never block - pure prefetch),
ctx/out DMAs on GpSimd (SWDGE), evictions + normalization on DVE,
exp on ACT.  PSUM (8 banks): "pj" 2, "sc" 2x[128,1024]=4, "ctx" 2.
"""

import sys

sys.path.insert(0, "/opt/trn_rl_repo")

from contextlib import ExitStack

import ml_dtypes
import numpy as np

import concourse.bass as bass
import concourse.tile as tile
from concourse import bacc, mybir
from concourse.bass_utils import run_bass_kernel_spmd

F32 = mybir.dt.float32
F32R = mybir.dt.float32r
BF16 = mybir.dt.bfloat16

B = 2
S = 2048
D = 1024
H = 16
DK = 64
SCALING = 2.0
N_CORES = 8
CPG = 4
CSLICE = D // CPG
Exp = mybir.ActivationFunctionType.Exp
MULT = mybir.AluOpType.mult

_CACHE = {}


def _build():
    nc = bacc.Bacc("TRN2", target_bir_lowering=False, debug=False)

    xqT = nc.declare_dram_parameter("xqT", [D, S], BF16, isOutput=False)
    xkT = nc.declare_dram_parameter("xkT", [D, S], BF16, isOutput=False)
    xvT = nc.declare_dram_parameter("xvT", [D, S], BF16, isOutput=False)
    wq = nc.declare_dram_parameter("wq", [D, CSLICE], BF16, isOutput=False)
    wk = nc.declare_dram_parameter("wk", [D, CSLICE], BF16, isOutput=False)
    wv = nc.declare_dram_parameter("wv", [D, CSLICE], BF16, isOutput=False)
    wo = nc.declare_dram_parameter("wo", [CSLICE, D], BF16, isOutput=False)
    outT = nc.declare_dram_parameter("outT", [D, S], F32, isOutput=True)

    with tile.TileContext(nc) as tc, ExitStack() as ctx:
        const = ctx.enter_context(tc.tile_pool(name="const", bufs=1))
        xp = ctx.enter_context(tc.tile_pool(name="xp", bufs=16))
        expp = ctx.enter_context(tc.tile_pool(name="expp", bufs=6))
        smallp = ctx.enter_context(tc.tile_pool(name="smallp", bufs=2))
        psum = ctx.enter_context(tc.tile_pool(name="psum", bufs=2, space="PSUM"))

        wq_sb = const.tile([128, 8, CSLICE], BF16)
        wk_sb = const.tile([128, 8, CSLICE], BF16)
        wv_sb = const.tile([128, 8, CSLICE], BF16)
        wo_sb = const.tile([128, 2, D], BF16)
        nc.sync.dma_start(wk_sb[:], wk.rearrange("(i p) c -> p i c", p=128))
        nc.sync.dma_start(wq_sb[:], wq.rearrange("(i p) c -> p i c", p=128))
        nc.sync.dma_start(wv_sb[:], wv.rearrange("(i p) c -> p i c", p=128))
        nc.sync.dma_start(wo_sb[:], wo.rearrange("(c p) o -> p c o", p=128))

        kT_s = [const.tile([128, 2, 512], BF16, name=f"kT{i}") for i in range(4)]
        qT_s = [const.tile([128, 2, 512], BF16, name=f"qT{i}") for i in range(4)]
        v_s = [const.tile([128, 4, 4, DK + 1], BF16, name=f"v{i}") for i in range(4)]
        ctxT_s = [const.tile([128, 2, 512], BF16, name=f"cx{i}") for i in range(4)]

        ones_f = const.tile([128, 16], F32)
        nc.vector.memset(ones_f[:], 1.0)
        for tt in range(4):
            nc.vector.tensor_copy(
                v_s[tt][:, :, :, DK : DK + 1],
                ones_f[:].rearrange("p (a b c) -> p a b c", a=4, b=4, c=1),
            )

        def proj_load(xsrc, st):
            xts = []
            for i in range(8):
                xt = xp.tile([128, 512], BF16, tag="xt")
                nc.sync.dma_start(
                    xt[:], xsrc[128 * i : 128 * (i + 1), 512 * st : 512 * (st + 1)]
                )
                xts.append(xt)
            return xts

        def proj_round(wsb, dst_tile, xts, cc):
            ps = psum.tile([128, 512], F32, tag="pj", bufs=2)
            for i in range(8):
                nc.tensor.matmul(
                    ps[:],
                    wsb[:, i, 128 * cc : 128 * (cc + 1)],
                    xts[i][:],
                    start=(i == 0),
                    stop=(i == 7),
                )
            nc.vector.tensor_copy(dst_tile[:, cc, :], ps[:])

        def proj_qk(xsrc, wsb, dst_tile, st):
            xts = proj_load(xsrc, st)
            for cc in range(2):
                proj_round(wsb, dst_tile, xts, cc)

        # ---- projections: K, Q(first tile), V; Q(rest) interleaves into
        # ---- the attention stream as PE filler ---------------------------
        for st in range(4):
            proj_qk(xkT, wk_sb, kT_s[st], st)
        proj_qk(xqT, wq_sb, qT_s[0], 0)
        for tt in range(4):
            xts = []
            for i in range(8):
                xt = xp.tile([128, 512], BF16, tag="xt")
                nc.sync.dma_start(
                    xt[:], xvT[128 * i : 128 * (i + 1), 512 * tt : 512 * (tt + 1)]
                )
                xts.append(xt)
            for t4 in range(4):
                ps = psum.tile([128, 256], F32, tag="pj", bufs=2)
                for i in range(8):
                    nc.tensor.matmul(
                        ps[:],
                        xts[i][:, 128 * t4 : 128 * (t4 + 1)],
                        wv_sb[:, i, :],
                        start=(i == 0),
                        stop=(i == 7),
                    )
                nc.vector.tensor_copy(
                    v_s[tt][:, t4, :, 0:DK],
                    ps[:].rearrange("p (h d) -> p h d", h=4),
                )

        # ---- attention; out-proj of qt-1 rides inside qt's p==1 loop ----
        ops_live = {}

        def outproj_emit(oqt, o, phase):
            """phase 0: start matmul; 1: stop matmul; 2: evict + store
            (emitted 2 t-steps after the stop so the DVE never stalls)."""
            if phase == 0:
                ops = psum.tile([128, 512], F32, tag="pj", bufs=2, name=f"op{oqt}_{o}")
                ops_live[(oqt, o)] = ops
                nc.tensor.matmul(
                    ops[:],
                    wo_sb[:, 0, 128 * o : 128 * (o + 1)],
                    ctxT_s[oqt][:, 0, :],
                    start=True,
                    stop=False,
                )
            elif phase == 1:
                ops = ops_live[(oqt, o)]
                nc.tensor.matmul(
                    ops[:],
                    wo_sb[:, 1, 128 * o : 128 * (o + 1)],
                    ctxT_s[oqt][:, 1, :],
                    start=False,
                    stop=True,
                )
            else:
                ops = ops_live.pop((oqt, o))
                ob = smallp.tile([128, 512], F32, tag="ob", bufs=3)
                nc.vector.tensor_copy(ob[:], ops[:])
                nc.sync.dma_start(
                    outT[128 * o : 128 * (o + 1), 512 * oqt : 512 * (oqt + 1)], ob[:]
                )

        qnext = [None]

        def attention(qt):
            qtile = qT_s[qt]
            for p in range(2):
                ctx0 = psum.tile([DK + 1, 512], F32, tag="ctx", bufs=2)
                ctx1 = psum.tile([DK + 1, 512], F32, tag="ctx", bufs=2)
                for t in range(16):
                    sc = psum.tile([128, 1024], F32, tag="sc", bufs=2)
                    kt = kT_s[t // 4]
                    ts_ = slice(128 * (t % 4), 128 * (t % 4 + 1))
                    nc.tensor.matmul(
                        sc[:, 0:512],
                        kt[0:64, p, ts_],
                        qtile[0:64, p, :],
                        start=True,
                        stop=True,
                        tile_position=(0, 0),
                    )
                    nc.tensor.matmul(
                        sc[:, 512:1024],
                        kt[64:128, p, ts_],
                        qtile[64:128, p, :],
                        start=True,
                        stop=True,
                        tile_position=(64, 0),
                    )
                    et = expp.tile([128, 1024], BF16)
                    nc.scalar.activation(et[:], sc[:], Exp, scale=1.0 / 8.0)
                    nc.tensor.matmul(
                        ctx0[:],
                        v_s[t // 4][:, t % 4, 2 * p, :],
                        et[:, 0:512],
                        start=(t == 0),
                        stop=(t == 15),
                    )
                    nc.tensor.matmul(
                        ctx1[:],
                        v_s[t // 4][:, t % 4, 2 * p + 1, :],
                        et[:, 512:1024],
                        start=(t == 0),
                        stop=(t == 15),
                    )
                    if qt > 0:
                        ob_base = 4 * p
                        if t in (4, 6, 8, 10):
                            outproj_emit(qt - 1, ob_base + (t - 4) // 2, 0)
                        if t in (5, 7, 9, 11):
                            outproj_emit(qt - 1, ob_base + (t - 5) // 2, 1)
                        if t in (7, 9, 11, 13):
                            outproj_emit(qt - 1, ob_base + (t - 7) // 2, 2)
                    if p == 0 and qt < 3:
                        if t == 3:
                            qnext[0] = proj_load(xqT, qt + 1)
                        elif t == 7:
                            proj_round(wq_sb, qT_s[qt + 1], qnext[0], 0)
                        elif t == 11:
                            proj_round(wq_sb, qT_s[qt + 1], qnext[0], 1)
                # evict both ctx psums FIRST (fast PSUM release keeps the
                # PE fed; a >3.4us PE idle re-throttles the HAM clock gate),
                # then run recip/bcast/mult off the critical path.
                css = []
                for cx in (ctx0, ctx1):
                    cs = smallp.tile([DK + 1, 512], F32, tag="cs", bufs=4)
                    nc.vector.tensor_copy(cs[:], cx[:])
                    css.append(cs)
                # normalization: cheap approx reciprocal (base-0 input),
                # gpsimd broadcast, DVE multiply - off the PE critical path.
                for h01, cs in enumerate(css):
                    rs1 = smallp.tile([1, 512], F32, tag="rs1", bufs=3)
                    nc.vector.tensor_copy(rs1[:], cs[DK : DK + 1, :])
                    rc = smallp.tile([1, 512], F32, tag="rc", bufs=3)
                    nc.vector.reciprocal_approx_fast(rc[:], rs1[:])
                    bc = smallp.tile([64, 512], F32, tag="bc", bufs=3)
                    nc.gpsimd.partition_broadcast(bc[:], rc[:])
                    ct = smallp.tile([64, 512], BF16, tag="ct", bufs=3)
                    nc.vector.tensor_tensor(ct[:], cs[0:DK, :], bc[:], MULT)
                    nc.sync.dma_start(ctxT_s[qt][64 * h01 : 64 * h01 + 64, p, :], ct[:])

        for qt in range(4):
            attention(qt)
        # tail: out-proj of the last q-tile
        for o in range(8):
            outproj_emit(3, o, 0)
            outproj_emit(3, o, 1)
        for o in range(8):
            outproj_emit(3, o, 2)

    nc.finalize()
    return nc


def _get_nc():
    if "nc" not in _CACHE:
        _CACHE["nc"] = _build()
    return _CACHE["nc"]


def _numpy_reference(query, key, value, mask, Wq, Aq, Bq, Wk, Ak, Bk, Wv, Av, Bv, Wo, Ao, Bo):
    """Exact fallback for a non-all-ones mask (never hit for the spec'd inputs)."""

    def lora(x, W, A, Bm):
        return x @ W.T + ((x @ A.T) @ Bm.T) * SCALING

    q = lora(query, Wq, Aq, Bq).reshape(B, S, H, DK).transpose(0, 2, 1, 3)
    k = lora(key, Wk, Ak, Bk).reshape(B, S, H, DK).transpose(0, 2, 1, 3)
    v = lora(value, Wv, Av, Bv).reshape(B, S, H, DK).transpose(0, 2, 1, 3)
    sc = np.einsum("bhqd,bhkd->bhqk", q, k) / np.sqrt(np.float32(DK))
    sc = np.where(mask == 0, np.float32(-1e9), sc)
    sc = sc - sc.max(axis=-1, keepdims=True)
    e = np.exp(sc)
    attn = e / e.sum(axis=-1, keepdims=True)
    cx = np.einsum("bhqk,bhkd->bhqd", attn, v)
    cx = cx.transpose(0, 2, 1, 3).reshape(B, S, D)
    return lora(cx, Wo, Ao, Bo).astype(np.float32)


def _prepare_in_maps(query, key, value, Wq, Aq, Bq, Wk, Ak, Bk, Wv, Av, Bv, Wo, Ao, Bo):
    f32 = np.float32
    bf16 = ml_dtypes.bfloat16
    weff = {}
    for n, (W, A, Bm) in {
        "q": (Wq, Aq, Bq),
        "k": (Wk, Ak, Bk),
        "v": (Wv, Av, Bv),
        "o": (Wo, Ao, Bo),
    }.items():
        weff[n] = (
            np.asarray(W, f32) + SCALING * np.asarray(Bm, f32) @ np.asarray(A, f32)
        ).astype(f32)

    xT = {
        "q": [np.ascontiguousarray(np.asarray(query[b], f32).T).astype(bf16) for b in range(B)],
        "k": [np.ascontiguousarray(np.asarray(key[b], f32).T).astype(bf16) for b in range(B)],
        "v": [np.ascontiguousarray(np.asarray(value[b], f32).T).astype(bf16) for b in range(B)],
    }
    in_maps = []
    for c in range(N_CORES):
        b, g = divmod(c, CPG)
        cs = slice(CSLICE * g, CSLICE * (g + 1))
        in_maps.append(
            {
                "xqT": xT["q"][b],
                "xkT": xT["k"][b],
                "xvT": xT["v"][b],
                "wq": np.ascontiguousarray(weff["q"][cs, :].T).astype(bf16),
                "wk": np.ascontiguousarray(weff["k"][cs, :].T).astype(bf16),
                "wv": np.ascontiguousarray(weff["v"][cs, :].T).astype(bf16),
                "wo": np.ascontiguousarray(weff["o"][:, cs].T).astype(bf16),
            }
        )
    return in_maps


def run(inputs, trace=False, **spmd_kwargs):
    """Shard, run on 8 cores, gather.  Returns (output, BassKernelResults)."""
    mask = np.asarray(inputs["mask"])
    if not np.all(mask != 0):
        out = _numpy_reference(
            np.asarray(inputs["query"], np.float32),
            np.asarray(inputs["key"], np.float32),
            np.asarray(inputs["value"], np.float32),
            mask,
            *[
                np.asarray(inputs[k], np.float32)
                for k in ("Wq", "Aq", "Bq", "Wk", "Ak", "Bk", "Wv", "Av", "Bv", "Wo", "Ao", "Bo")
            ],
        )
        return out, None

    in_maps = _prepare_in_maps(
        inputs["query"], inputs["key"], inputs["value"],
        inputs["Wq"], inputs["Aq"], inputs["Bq"],
        inputs["Wk"], inputs["Ak"], inputs["Bk"],
        inputs["Wv"], inputs["Av"], inputs["Bv"],
        inputs["Wo"], inputs["Ao"], inputs["Bo"],
    )
    nc = _get_nc()
    res = run_bass_kernel_spmd(
        nc, in_maps, core_ids=list(range(N_CORES)), trace=trace, **spmd_kwargs
    )
    out = np.empty((B, S, D), np.float32)
    for b in range(B):
        acc = res.results[CPG * b]["outT"].astype(np.float32)
        for g in range(1, CPG):
            acc = acc + res.results[CPG * b + g]["outT"]
        out[b] = acc.T
    return out, res


def kernel(**inputs):
    out, _ = run(inputs, trace=False)
    return out


# revision 26
# speedup vs baseline: 1.2212x; 1.0151x over previous
"""LoRA multi-head attention on 8 Trainium2 NeuronCores.

Sharding: data-parallel over batch (B=2) x tensor-parallel over heads
(16 heads -> 4 per core).  Core c handles batch b=c//4 and head group
g=c%4 (columns C=[256*g, 256*g+256) of the projection output).

Host prep (per weight): W_eff = W + 2.0 * B @ A  (exact LoRA fold),
and transposed activations x.T so the contraction dim lands on SBUF
partitions.  x and the QKV weights ship as bf16 (halves the DMA
prefix); attention tensors stay float32r (fp32 storage, full-rate PE).

Device, per core:
  K^T [256,2048], Q^T, V [2048,256] (per-head 65-wide, ones column
  folded in for softmax row sums), then per 512-wide q-tile x head
  pair: scoresT [t,q] (row-packed pair, one PSUM tile) -> exp (ACT,
  scale=1/8) -> PV -> pair-fused normalization -> ctx^T -> partial
  output projection (interleaved into the NEXT q-tile's PE stream;
  summed on host across the batch group).

Engine routing: input DMAs on Sync (never block - pure prefetch),
ctx/out DMAs on GpSimd (SWDGE), evictions + normalization on DVE,
exp on ACT.  PSUM (8 banks): "pj" 2, "sc" 2x[128,1024]=4, "ctx" 2.
"""

import sys

sys.path.insert(0, "/opt/trn_rl_repo")

from contextlib import ExitStack

import ml_dtypes
import numpy as np

import concourse.bass as bass
import concourse.tile as tile
from concourse import bacc, mybir
from concourse.bass_utils import run_bass_kernel_spmd

F32 = mybir.dt.float32
F32R = mybir.dt.float32r
BF16 = mybir.dt.bfloat16

B = 2
S = 2048
D = 1024
H = 16
DK = 64
SCALING = 2.0
N_CORES = 8
CPG = 4
CSLICE = D // CPG
Exp = mybir.ActivationFunctionType.Exp
MULT = mybir.AluOpType.mult

_CACHE = {}


def _build():
    nc = bacc.Bacc("TRN2", target_bir_lowering=False, debug=False)

    xqT = nc.declare_dram_parameter("xqT", [D, S], BF16, isOutput=False)
    xkT = nc.declare_dram_parameter("xkT", [D, S], BF16, isOutput=False)
    xvT = nc.declare_dram_parameter("xvT", [D, S], BF16, isOutput=False)
    wq = nc.declare_dram_parameter("wq", [D, CSLICE], BF16, isOutput=False)
    wk = nc.declare_dram_parameter("wk", [D, CSLICE], BF16, isOutput=False)
    wv = nc.declare_dram_parameter("wv", [D, CSLICE], BF16, isOutput=False)
    wo = nc.declare_dram_parameter("wo", [CSLICE, D], BF16, isOutput=False)
    outT = nc.declare_dram_parameter("outT", [D, S], F32, isOutput=True)

    with tile.TileContext(nc) as tc, ExitStack() as ctx:
        const = ctx.enter_context(tc.tile_pool(name="const", bufs=1))
        xp = ctx.enter_context(tc.tile_pool(name="xp", bufs=16))
        expp = ctx.enter_context(tc.tile_pool(name="expp", bufs=6))
        smallp = ctx.enter_context(tc.tile_pool(name="smallp", bufs=2))
        psum = ctx.enter_context(tc.tile_pool(name="psum", bufs=2, space="PSUM"))

        wq_sb = const.tile([128, 8, CSLICE], BF16)
        wk_sb = const.tile([128, 8, CSLICE], BF16)
        wv_sb = const.tile([128, 8, CSLICE], BF16)
        wo_sb = const.tile([128, 2, D], BF16)
        nc.sync.dma_start(wk_sb[:], wk.rearrange("(i p) c -> p i c", p=128))
        nc.sync.dma_start(wq_sb[:], wq.rearrange("(i p) c -> p i c", p=128))
        nc.sync.dma_start(wv_sb[:], wv.rearrange("(i p) c -> p i c", p=128))
        nc.sync.dma_start(wo_sb[:], wo.rearrange("(c p) o -> p c o", p=128))

        kT_s = [const.tile([128, 2, 512], BF16, name=f"kT{i}") for i in range(4)]
        qT_s = [const.tile([128, 2, 512], BF16, name=f"qT{i}") for i in range(4)]
        v_s = [const.tile([128, 4, 4, DK + 1], BF16, name=f"v{i}") for i in range(4)]
        ctxT_s = [const.tile([128, 2, 512], BF16, name=f"cx{i}") for i in range(4)]

        ones_f = const.tile([128, 16], F32)
        nc.vector.memset(ones_f[:], 1.0)
        for tt in range(4):
            nc.vector.tensor_copy(
                v_s[tt][:, :, :, DK : DK + 1],
                ones_f[:].rearrange("p (a b c) -> p a b c", a=4, b=4, c=1),
            )

        def proj_load(xsrc, st):
            xts = []
            for i in range(8):
                xt = xp.tile([128, 512], BF16, tag="xt")
                nc.sync.dma_start(
                    xt[:], xsrc[128 * i : 128 * (i + 1), 512 * st : 512 * (st + 1)]
                )
                xts.append(xt)
            return xts

        def proj_round(wsb, dst_tile, xts, cc):
            ps = psum.tile([128, 512], F32, tag="pj", bufs=2)
            for i in range(8):
                nc.tensor.matmul(
                    ps[:],
                    wsb[:, i, 128 * cc : 128 * (cc + 1)],
                    xts[i][:],
                    start=(i == 0),
                    stop=(i == 7),
                )
            nc.vector.tensor_copy(dst_tile[:, cc, :], ps[:])

        def proj_qk(xsrc, wsb, dst_tile, st):
            xts = proj_load(xsrc, st)
            for cc in range(2):
                proj_round(wsb, dst_tile, xts, cc)

        # ---- projections: K then Q(first tile).  V rounds are folded into
        # ---- the first attention t-loop (PV lags 4 steps, exactly the
        # ---- per-512-row V completion cadence) ---------------------------
        for st in range(4):
            proj_qk(xkT, wk_sb, kT_s[st], st)
        proj_qk(xqT, wq_sb, qT_s[0], 0)

        def v_round(xts, tt, t4):
            ps = psum.tile([128, 256], F32, tag="pj", bufs=2, name=f"vps{tt}_{t4}")
            for i in range(8):
                nc.tensor.matmul(
                    ps[:],
                    xts[i][:, 128 * t4 : 128 * (t4 + 1)],
                    wv_sb[:, i, :],
                    start=(i == 0),
                    stop=(i == 7),
                )
            nc.vector.tensor_copy(
                v_s[tt][:, t4, :, 0:DK],
                ps[:].rearrange("p (h d) -> p h d", h=4),
            )

        def v_load(tt):
            xts = []
            for i in range(8):
                xt = xp.tile([128, 512], BF16, tag="xv", bufs=16, name=f"xv{tt}_{i}")
                nc.sync.dma_start(
                    xt[:],
                    xvT[128 * i : 128 * (i + 1), 512 * tt : 512 * (tt + 1)],
                )
                xts.append(xt)
            return xts

        # ---- attention; out-proj of qt-1 rides inside qt's p==1 loop ----
        ops_live = {}

        def outproj_emit(oqt, o, phase):
            """phase 0: start matmul; 1: stop matmul; 2: evict + store
            (emitted 2 t-steps after the stop so the DVE never stalls)."""
            if phase == 0:
                ops = psum.tile([128, 512], F32, tag="pj", bufs=2, name=f"op{oqt}_{o}")
                ops_live[(oqt, o)] = ops
                nc.tensor.matmul(
                    ops[:],
                    wo_sb[:, 0, 128 * o : 128 * (o + 1)],
                    ctxT_s[oqt][:, 0, :],
                    start=True,
                    stop=False,
                )
            elif phase == 1:
                ops = ops_live[(oqt, o)]
                nc.tensor.matmul(
                    ops[:],
                    wo_sb[:, 1, 128 * o : 128 * (o + 1)],
                    ctxT_s[oqt][:, 1, :],
                    start=False,
                    stop=True,
                )
            else:
                ops = ops_live.pop((oqt, o))
                ob = smallp.tile([128, 512], F32, tag="ob", bufs=3)
                nc.vector.tensor_copy(ob[:], ops[:])
                nc.sync.dma_start(
                    outT[128 * o : 128 * (o + 1), 512 * oqt : 512 * (oqt + 1)], ob[:]
                )

        qnext = [None]

        def attention(qt):
            qtile = qT_s[qt]
            vfold = qt == 0

            def scores_exp(p, t):
                sc = psum.tile([128, 1024], F32, tag="sc", bufs=2, name=f"sc{qt}{p}{t}")
                kt = kT_s[t // 4]
                ts_ = slice(128 * (t % 4), 128 * (t % 4 + 1))
                nc.tensor.matmul(
                    sc[:, 0:512],
                    kt[0:64, p, ts_],
                    qtile[0:64, p, :],
                    start=True,
                    stop=True,
                    tile_position=(0, 0),
                )
                nc.tensor.matmul(
                    sc[:, 512:1024],
                    kt[64:128, p, ts_],
                    qtile[64:128, p, :],
                    start=True,
                    stop=True,
                    tile_position=(64, 0),
                )
                et = expp.tile([128, 1024], BF16, tag="et", bufs=8, name=f"et{qt}{p}{t}")
                nc.scalar.activation(et[:], sc[:], Exp, scale=1.0 / 8.0)
                return et

            def pv(ctx0, ctx1, p, t, et):
                nc.tensor.matmul(
                    ctx0[:],
                    v_s[t // 4][:, t % 4, 2 * p, :],
                    et[:, 0:512],
                    start=(t == 0),
                    stop=(t == 15),
                )
                nc.tensor.matmul(
                    ctx1[:],
                    v_s[t // 4][:, t % 4, 2 * p + 1, :],
                    et[:, 512:1024],
                    start=(t == 0),
                    stop=(t == 15),
                )

            for p in range(2):
                ctx0 = psum.tile([DK + 1, 512], F32, tag="ctx", bufs=2)
                ctx1 = psum.tile([DK + 1, 512], F32, tag="ctx", bufs=2)
                ets = {}
                for t in range(16):
                    if vfold and p == 0:
                        # V projection rides in this stream; PV lags 4 steps
                        # (group g of v completes at step 4g+3).
                        if t % 4 == 0:
                            vx = v_load(t // 4)
                        v_round(vx, t // 4, t % 4)
                        ets[t] = scores_exp(p, t)
                        if t >= 4:
                            pv(ctx0, ctx1, p, t - 4, ets.pop(t - 4))
                    else:
                        et = scores_exp(p, t)
                        pv(ctx0, ctx1, p, t, et)
                    if qt > 0:
                        ob_base = 4 * p
                        if t in (4, 6, 8, 10):
                            outproj_emit(qt - 1, ob_base + (t - 4) // 2, 0)
                        if t in (5, 7, 9, 11):
                            outproj_emit(qt - 1, ob_base + (t - 5) // 2, 1)
                        if t in (7, 9, 11, 13):
                            outproj_emit(qt - 1, ob_base + (t - 7) // 2, 2)
                    if p == 0 and qt < 3:
                        if t == 3:
                            qnext[0] = proj_load(xqT, qt + 1)
                        elif t == 7:
                            proj_round(wq_sb, qT_s[qt + 1], qnext[0], 0)
                        elif t == 11:
                            proj_round(wq_sb, qT_s[qt + 1], qnext[0], 1)
                for tl in sorted(ets):
                    pv(ctx0, ctx1, p, tl, ets.pop(tl))
                # evict both ctx psums FIRST (fast PSUM release keeps the
                # PE fed; a >3.4us PE idle re-throttles the HAM clock gate),
                # then run recip/bcast/mult off the critical path.
                css = []
                for cx in (ctx0, ctx1):
                    cs = smallp.tile([DK + 1, 512], F32, tag="cs", bufs=4)
                    nc.vector.tensor_copy(cs[:], cx[:])
                    css.append(cs)
                # normalization: cheap approx reciprocal (base-0 input),
                # gpsimd broadcast, DVE multiply - off the PE critical path.
                for h01, cs in enumerate(css):
                    rs1 = smallp.tile([1, 512], F32, tag="rs1", bufs=3)
                    nc.vector.tensor_copy(rs1[:], cs[DK : DK + 1, :])
                    rc = smallp.tile([1, 512], F32, tag="rc", bufs=3)
                    nc.vector.reciprocal_approx_fast(rc[:], rs1[:])
                    bc = smallp.tile([64, 512], F32, tag="bc", bufs=3)
                    nc.gpsimd.partition_broadcast(bc[:], rc[:])
                    ct = smallp.tile([64, 512], BF16, tag="ct", bufs=3)
                    nc.vector.tensor_tensor(ct[:], cs[0:DK, :], bc[:], MULT)
                    nc.sync.dma_start(ctxT_s[qt][64 * h01 : 64 * h01 + 64, p, :], ct[:])

        for qt in range(4):
            attention(qt)
        # tail: out-proj of the last q-tile
        for o in range(8):
            outproj_emit(3, o, 0)
            outproj_emit(3, o, 1)
        for o in range(8):
            outproj_emit(3, o, 2)

    nc.finalize()
    return nc


def _get_nc():
    if "nc" not in _CACHE:
        _CACHE["nc"] = _build()
    return _CACHE["nc"]


def _numpy_reference(query, key, value, mask, Wq, Aq, Bq, Wk, Ak, Bk, Wv, Av, Bv, Wo, Ao, Bo):
    """Exact fallback for a non-all-ones mask (never hit for the spec'd inputs)."""

    def lora(x, W, A, Bm):
        return x @ W.T + ((x @ A.T) @ Bm.T) * SCALING

    q = lora(query, Wq, Aq, Bq).reshape(B, S, H, DK).transpose(0, 2, 1, 3)
    k = lora(key, Wk, Ak, Bk).reshape(B, S, H, DK).transpose(0, 2, 1, 3)
    v = lora(value, Wv, Av, Bv).reshape(B, S, H, DK).transpose(0, 2, 1, 3)
    sc = np.einsum("bhqd,bhkd->bhqk", q, k) / np.sqrt(np.float32(DK))
    sc = np.where(mask == 0, np.float32(-1e9), sc)
    sc = sc - sc.max(axis=-1, keepdims=True)
    e = np.exp(sc)
    attn = e / e.sum(axis=-1, keepdims=True)
    cx = np.einsum("bhqk,bhkd->bhqd", attn, v)
    cx = cx.transpose(0, 2, 1, 3).reshape(B, S, D)
    return lora(cx, Wo, Ao, Bo).astype(np.float32)


def _prepare_in_maps(query, key, value, Wq, Aq, Bq, Wk, Ak, Bk, Wv, Av, Bv, Wo, Ao, Bo):
    f32 = np.float32
    bf16 = ml_dtypes.bfloat16
    weff = {}
    for n, (W, A, Bm) in {
        "q": (Wq, Aq, Bq),
        "k": (Wk, Ak, Bk),
        "v": (Wv, Av, Bv),
        "o": (Wo, Ao, Bo),
    }.items():
        weff[n] = (
            np.asarray(W, f32) + SCALING * np.asarray(Bm, f32) @ np.asarray(A, f32)
        ).astype(f32)

    xT = {
        "q": [np.ascontiguousarray(np.asarray(query[b], f32).T).astype(bf16) for b in range(B)],
        "k": [np.ascontiguousarray(np.asarray(key[b], f32).T).astype(bf16) for b in range(B)],
        "v": [np.ascontiguousarray(np.asarray(value[b], f32).T).astype(bf16) for b in range(B)],
    }
    in_maps = []
    for c in range(N_CORES):
        b, g = divmod(c, CPG)
        cs = slice(CSLICE * g, CSLICE * (g + 1))
        in_maps.append(
            {
                "xqT": xT["q"][b],
                "xkT": xT["k"][b],
                "xvT": xT["v"][b],
                "wq": np.ascontiguousarray(weff["q"][cs, :].T).astype(bf16),
                "wk": np.ascontiguousarray(weff["k"][cs, :].T).astype(bf16),
                "wv": np.ascontiguousarray(weff["v"][cs, :].T).astype(bf16),
                "wo": np.ascontiguousarray(weff["o"][:, cs].T).astype(bf16),
            }
        )
    return in_maps


def run(inputs, trace=False, **spmd_kwargs):
    """Shard, run on 8 cores, gather.  Returns (output, BassKernelResults)."""
    mask = np.asarray(inputs["mask"])
    if not np.all(mask != 0):
        out = _numpy_reference(
            np.asarray(inputs["query"], np.float32),
            np.asarray(inputs["key"], np.float32),
            np.asarray(inputs["value"], np.float32),
            mask,
            *[
                np.asarray(inputs[k], np.float32)
                for k in ("Wq", "Aq", "Bq", "Wk", "Ak", "Bk", "Wv", "Av", "Bv", "Wo", "Ao", "Bo")
            ],
        )
        return out, None

    in_maps = _prepare_in_maps(
        inputs["query"], inputs["key"], inputs["value"],
        inputs["Wq"], inputs["Aq"], inputs["Bq"],
        inputs["Wk"], inputs["Ak"], inputs["Bk"],
        inputs["Wv"], inputs["Av"], inputs["Bv"],
        inputs["Wo"], inputs["Ao"], inputs["Bo"],
    )
    nc = _get_nc()
    res = run_bass_kernel_spmd(
        nc, in_maps, core_ids=list(range(N_CORES)), trace=trace, **spmd_kwargs
    )
    out = np.empty((B, S, D), np.float32)
    for b in range(B):
        acc = res.results[CPG * b]["outT"].astype(np.float32)
        for g in range(1, CPG):
            acc = acc + res.results[CPG * b + g]["outT"]
        out[b] = acc.T
    return out, res


def kernel(**inputs):
    out, _ = run(inputs, trace=False)
    return out
